# revision 38
# baseline (speedup 1.0000x reference)
"""Trainium2 Bass kernel for: out = conv3x3(x, weight*A_w) * sigmoid(conv3x3(relu(conv3x3(x, se_w1)), se_w2))

Sharding: data-parallel over batch B=8 -> 8 NeuronCores (one image per core);
weights replicated. A_w is folded into the conv weight on the host (f32
multiply, then bf16 cast), so the device sees one fused weight tensor.

Per-core kernel (direct conv as implicit GEMM on the TensorEngine):
  - x stored column-padded [ci, 56, 58] bf16 in SBUF (pad cols zeroed,
    +1-element guards at both flat ends) so every 3x3 tap is a contiguous
    1-D shifted window (the matmul ISA requires single-free-dim operands).
  - row taps at the image top/bottom use clipped row ranges; the center tap
    is issued first per ci-block pass (full coverage, start=True), the
    clipped taps accumulate -> exact zero-padding semantics.
  - compute dtype bf16 (fp32 PSUM accumulate), rel-err vs fp32 ~3e-3.
  - x DMA is row-chunked (4 chunks, one 3-D trigger each covering both
    ci-blocks) and ordered ahead of the big main-conv weight on the sync
    queue; startup is DMA-bandwidth-bound, so the critical first deps (SE
    w1 + chunk A) get the gpsimd + sync queues while everything else
    queues behind in need-order.
  - SE kw-strip reduction m = s0(<<1) + s1 + s2(>>1) runs as a ScalarE
    strip copy + two shifted scalar_tensor_tensor adds on the DVE straight
    out of PSUM — the conv1 selector matmul is gone (the conv2 selector
    matmul remains: it also replicates the single attention channel to all
    128 partitions via a K=16 identity matmul).
  - schedule: main groups interleave with the SE chain from the VERY first
    step (an SE-only warm-up phase stalls the PE ~4us on PSUM-drain
    latencies): the c=0 groups for tiles 0-5 run RAW (conv drained to SBUF
    by ScalarE before any attention exists; a vector mulfix multiplies the
    attention in later and streams the tile out), tiles 4/5 split into row
    halves to fill the two late-phase slots with no sigmoided attention
    yet; every other main is fused (attention multiply straight out of
    PSUM + per-tile output DMA). Each dense 18-MM main (~3.5us) separates
    every SE producer from its consumer.
  - ScalarE activation tables (identity/relu/sigmoid) are pre-warmed with
    dummy activations during the DMA wait.
  - the last main group splits its mul+DMA into row halves on both queues
    to shorten the output tail.

Measured on HW: baseline-fused-schedule 86.6us -> this version ~84.0us.
Fixed costs: ~5.9us engine preamble before the measured window, ~5us
startup (DMA trigger latency ~1.4us + ~1.3MB critical input stream),
~11.4us tail after the last matmul (mul+DMA ~2 + framework teardown
barriers ~9, invariant to scheduling). PE floor (bf16 implicit GEMM,
1 moving column/cycle @2.4GHz): ~62us.

Rejected variants (measured slower): SE-branch-first scheduling (exposes
the serial PSUM-drain chain, 91us); 1-D F(2,3) Winograd main conv (24 MMs
of free 392 per 14-row tile instead of 36 of 464 per 16 rows, but the
7 extra [128,392] elementwise ops per group saturate Vector/Scalar, 101us);
spreading the startup loads across the scalar/gpsimd DMA queues in
parallel (queues split bandwidth evenly, starving whichever chunk is
needed next: +2..4us in three different arrangements); fp8 (quantization
noise ~5% exceeds the 2e-2 gate); packing kh into the SE stationary
(drain-strip count triples, saturating the DVE).
"""

import numpy as np

import concourse.bass as bass  # noqa: F401
import concourse.mybir as mybir
import concourse.tile as tile
from concourse import bacc
from concourse.bass_utils import run_bass_kernel_spmd

B, C, H, W = 8, 256, 56, 56
HW = H * W
WP = W + 2                      # padded row width (c=0 left pad, c=57 right pad)
HWP = H * WP                    # 3248
CMID = 16
N_CORES = 8
RT = 8                          # output rows per PSUM tile
NT = H // RT                    # 7
F32 = mybir.dt.float32
BF16 = mybir.dt.bfloat16

# x DMA row-chunk boundaries: chunk A rows [0,10) serves SE tile 0,
# B rows [10,18) tile 1, C rows [18,34) tiles 2-3, D rows [34,56) tiles 4-6.
CH_A = 10
CH_B = 18
CH_C = 34

# center tap first within each ci-block pass
TAPS = [(0, 0)] + [
    (dh, dw) for dh in (-1, 0, 1) for dw in (-1, 0, 1) if (dh, dw) != (0, 0)
]


def _rows(r0, dh):
    """Clipped local row range [rl, rh) of a tile at base row r0 for row-tap dh."""
    return max(0, -dh - r0), min(RT, H - dh - r0)


def build():
    nc = bacc.Bacc("TRN2", target_bir_lowering=False, debug=False, num_devices=N_CORES)

    # x pre-padded on host: [128, ci-block * (1 + 56*58 + 1)] bf16 (both
    # ci-blocks side by side per partition) so each row-chunk loads with a
    # single 3-D DMA trigger; zero pad columns and flat-end guards baked in
    x_d = nc.dram_tensor("xpad", [128, 2 * (HWP + 2)], BF16, kind="ExternalInput").ap()
    # (weight * A_w) transposed on host, split by OUTPUT-channel block so
    # the c=0 weights (first raw main groups' dep) land first:
    # [co-block, 128ci, ci-block * 9 * 128co]
    wm_d = nc.dram_tensor(
        "wmodT", [2, 128, 2 * 9 * 128], BF16, kind="ExternalInput"
    ).ap()
    # SE weights pre-packed on host: w1 as one tensor (cols [0:288) block 0,
    # [288:576) block 1; kw groups at 32-col strides per kh), w2 separate
    w1_d = nc.dram_tensor("sew1P", [128, 2 * 288], BF16, kind="ExternalInput").ap()
    w2_d = nc.dram_tensor("sew2P", [CMID, 288], BF16, kind="ExternalInput").ap()
    # output in padded layout [ci-block, 128, 56*58]; host strips pad cols
    out_d = nc.dram_tensor("outp", [2, 128, HWP], F32, kind="ExternalOutput").ap()

    with tile.TileContext(nc) as tc:
        with (
            tc.tile_pool(name="sb", bufs=1) as sb,
            tc.tile_pool(name="ps", space="PSUM", bufs=2) as ps,
        ):
            asb = sb.tile([128, HWP], F32, name="asb")
            asig = sb.tile([CMID, RT * WP], F32, name="asig")
            osb = [sb.tile([128, HWP], F32, name=f"osb{c}") for c in range(2)]
            # +2: one guard element at each flat end (dw=+-1 at image corners)
            xsb = sb.tile([128, 2 * (HWP + 2)], BF16, name="xsb")
            xs = [xsb[:, i * (HWP + 2) : (i + 1) * (HWP + 2)] for i in range(2)]
            wmod = [sb.tile([128, 2 * 9 * 128], BF16, name=f"wmod{c}") for c in range(2)]
            mid = sb.tile([CMID, HWP + 2], BF16, name="mid")
            u1pp = [sb.tile([96, RT * WP], BF16, name=f"u1pp{k}") for k in range(2)]
            u2pp = [sb.tile([96, RT * WP], BF16, name=f"u2pp{k}") for k in range(2)]
            wsb = sb.tile([128, 2 * 288], BF16, name="wsb")
            w1pack = [wsb[:, i * 288 : (i + 1) * 288] for i in range(2)]
            w2sb = sb.tile([CMID, 288], BF16, name="w2sb")
            w2pack = w2sb[:, :]

            # -------- loads --------
            # One trigger per transfer (each DMA_DIRECT2D costs ~0.6us of
            # engine time; data starts ~1.4us after the trigger). Queue
            # ORDER is the priority mechanism: parallel queues split DMA
            # bandwidth evenly, so the critical first-matmul deps (w1 +
            # chunk A, ~0.45MB total) get three queues to themselves while
            # the bulk weights queue up behind them (measured: wmod sharing
            # the early window starves chunk A and delays the first matmul
            # by 8us). Need-order per queue; the tiny w2 rides gpsimd.
            fA = 1 + CH_A * WP
            fB = 1 + CH_B * WP
            fC = 1 + CH_C * WP
            xsb_r = xsb.rearrange("p (i f) -> p i f", i=2)
            x_d_r = x_d.rearrange("p (i f) -> p i f", i=2)
            nc.gpsimd.dma_start(wsb, w1_d)
            nc.gpsimd.dma_start(w2sb, w2_d)
            nc.sync.dma_start(xsb_r[:, :, 0:fA], x_d_r[:, :, 0:fA])
            nc.sync.dma_start(xsb_r[:, :, fA:fB], x_d_r[:, :, fA:fB])
            nc.sync.dma_start(wmod[0], wm_d[0])
            nc.sync.dma_start(xsb_r[:, :, fB:fC], x_d_r[:, :, fB:fC])
            nc.sync.dma_start(xsb_r[:, :, fC : HWP + 2], x_d_r[:, :, fC : HWP + 2])
            nc.sync.dma_start(wmod[1], wm_d[1])

            def pad_memset(tl, np_):
                nc.vector.memset(tl[:np_, 0:2], 0.0)
                nc.vector.memset(tl[:np_, HWP : HWP + 2], 0.0)
                pads = tl[:np_, 1 + W + 1 : 1 + W + 1 + (H - 1) * WP].rearrange(
                    "p (h c) -> p h c", c=WP
                )
                nc.vector.memset(pads[:, :, 0:2], 0.0)

            # -------- prep (VectorE only, no PE) --------
            # pre-warm ScalarE activation tables (sigmoid/relu/identity)
            # during the DMA wait so the ~1.3us table loads don't stall the
            # SE dependency chain mid-kernel
            warm = sb.tile([1, 2], F32, name="warm")
            nc.vector.memset(warm, 0.0)
            for fn in (
                mybir.ActivationFunctionType.Identity,
                mybir.ActivationFunctionType.Relu,
                mybir.ActivationFunctionType.Sigmoid,
            ):
                nc.scalar.activation(warm[0:1, 0:1], warm[0:1, 1:2], fn)
            pad_memset(mid, CMID)
            for k in range(2):
                nc.vector.memset(u1pp[k], 0.0)
                nc.vector.memset(u2pp[k], 0.0)

            mid_v = mid[:, 1 : 1 + HWP].rearrange("p (h c) -> p h c", c=WP)
            TFv = RT * WP
            wmod_v = [
                wmod[c].rearrange("p (i k co) -> p i k co", i=2, co=128)
                for c in range(2)
            ]

            # -------- conv group emitters --------
            # SE convs: the 3 kw taps are packed into the stationary columns
            # (48 = 3 kw x 16 ch). The kw-strip reduction m = s0(<<1) + s1 +
            # s2(>>1) runs as two shifted scalar_tensor_tensor adds on the
            # DVE straight out of PSUM (no selector matmul, no strip-drain
            # copies), keeping the PE and ScalarE out of the strip path.
            # Junk in pad columns only.
            ADD = mybir.AluOpType.add
            MUL = mybir.AluOpType.mult

            def strip_stt1(u, psrc):
                # the DVE allows only ONE PSUM operand per op: the middle
                # strip drains via ScalarE first (as before), then the stt
                # adds the +1-shifted s0 strip straight out of PSUM
                nc.scalar.activation(
                    u[32:48, :], psrc[32:48, :], mybir.ActivationFunctionType.Identity
                )
                nc.vector.scalar_tensor_tensor(
                    u[64:80, 1:TFv],
                    psrc[0:16, 0 : TFv - 1],
                    1.0,
                    u[32:48, 1:TFv],
                    MUL,
                    ADD,
                )

            def strip_stt2(u, psrc):
                # u[0:16] <- (s0<<1 + s1)  +  s2 shifted -1
                nc.vector.scalar_tensor_tensor(
                    u[0:16, 1 : TFv - 1],
                    u[64:80, 1 : TFv - 1],
                    1.0,
                    psrc[64:80, 2:TFv],
                    MUL,
                    ADD,
                )

            def conv1_pack(t):
                r0 = t * RT
                mps = ps.tile([96, TFv], F32, name="mps96", tag="pack", bufs=3)
                n_mm = 0
                for i in range(2):
                    for dh in (0, -1, 1):
                        kh = dh + 1
                        rl, rh = _rows(r0, dh)
                        n_mm += 1
                        nc.tensor.matmul(
                            mps[:, rl * WP : rh * WP],
                            w1pack[i][:, kh * 96 : (kh + 1) * 96],
                            xs[i][:, 1 + (r0 + rl + dh) * WP :][:128, : (rh - rl) * WP],
                            start=(n_mm == 1),
                            stop=(n_mm == 6),
                        )
                u = u1pp[t % 2]
                strip_stt1(u, mps)
                return u, mps

            def conv1_sel(t, u, mps):
                r0 = t * RT
                strip_stt2(u, mps)
                uv = u.rearrange("p (h c) -> p h c", c=WP)
                nc.scalar.activation(
                    mid_v[:, r0 : r0 + RT, 1 : W + 1],
                    uv[0:16, :, 1 : W + 1],
                    mybir.ActivationFunctionType.Relu,
                )

            def conv2_pack(t):
                r0 = t * RT
                ups = ps.tile([96, TFv], F32, name="u2ps", tag="pack", bufs=3)
                n_mm = 0
                for dh in (0, -1, 1):
                    kh = dh + 1
                    rl, rh = _rows(r0, dh)
                    n_mm += 1
                    nc.tensor.matmul(
                        ups[:, rl * WP : rh * WP],
                        w2pack[:, kh * 96 : (kh + 1) * 96],
                        mid[:, 1 + (r0 + rl + dh) * WP :][:CMID, : (rh - rl) * WP],
                        start=(n_mm == 1),
                        stop=(n_mm == 3),
                    )
                u = u2pp[t % 2]
                strip_stt1(u, ups)
                return u, ups

            def conv2_sel(t, u, ups):
                r0 = t * RT
                strip_stt2(u, ups)
                # sigmoid on the 16-partition strip, then the idle GpSimd
                # engine replicates the single attention channel to all 128
                # partitions (keeps the PE out of the attention path)
                nc.scalar.activation(
                    asig, u[0:16, :], mybir.ActivationFunctionType.Sigmoid
                )
                nc.gpsimd.partition_broadcast(
                    asb[:, r0 * WP : (r0 + RT) * WP], asig[0:1, :], channels=128
                )

            def main_mms(t, c, ra=0, rb=RT):
                r0 = t * RT
                yps = ps.tile([128, (rb - ra) * WP], F32, name="yps", tag="yps", bufs=3)
                n_mm = 0
                for i in range(2):
                    for dh, dw in TAPS:
                        k = (dh + 1) * 3 + (dw + 1)
                        rl = max(ra, -dh - r0)
                        rh = min(rb, H - dh - r0)
                        n_mm += 1
                        nc.tensor.matmul(
                            yps[:, (rl - ra) * WP : (rh - ra) * WP],
                            wmod_v[c][:, i, k, :],
                            xs[i][:, 1 + (r0 + rl + dh) * WP + dw :][:128, : (rh - rl) * WP],
                            start=(n_mm == 1),
                            stop=(n_mm == 18),
                        )
                return yps

            def main_fused(t, c, split=False):
                yps = main_mms(t, c)
                r0 = t * RT
                # the very last group splits its attention-mul + output DMA
                # into row halves on both queues so the final transfer is
                # half-size and overlaps the second mul (shorter tail)
                halves = ((0, RT // 2), (RT // 2, RT)) if split else ((0, RT),)
                for hi, (ra, rb) in enumerate(halves):
                    dst = osb[c][:, (r0 + ra) * WP : (r0 + rb) * WP]
                    amap = asb[:, (r0 + ra) * WP : (r0 + rb) * WP]
                    nc.vector.tensor_mul(dst, yps[:, ra * WP : rb * WP], amap)
                    q = nc.sync if (t + c + hi) % 2 == 0 else nc.scalar
                    q.dma_start(out_d[c][:, (r0 + ra) * WP : (r0 + rb) * WP], dst)

            def main_raw(t, c, ra=0, rb=RT):
                # conv only (attention map not yet available): drain the raw
                # conv result to SBUF on the scalar engine; mulfix() later
                # multiplies in the attention and streams the tile out
                yps = main_mms(t, c, ra, rb)
                r0 = t * RT
                nc.scalar.activation(
                    osb[c][:, (r0 + ra) * WP : (r0 + rb) * WP],
                    yps,
                    mybir.ActivationFunctionType.Identity,
                )

            def mulfix(t, c):
                r0 = t * RT
                dst = osb[c][:, r0 * WP : (r0 + RT) * WP]
                nc.vector.tensor_mul(dst, dst, asb[:, r0 * WP : (r0 + RT) * WP])
                q = nc.sync if (t + c) % 2 == 0 else nc.scalar
                q.dma_start(out_d[c][:, r0 * WP : (r0 + RT) * WP], dst)

            # -------- schedule ------------------------------------------
            # Main groups interleave with the SE chain from the very first
            # step (the SE chain alone stalls the PE ~4us on PSUM-drain
            # latencies): the six c=0 groups for tiles 0-5 run RAW (conv
            # drained to SBUF before any attention exists; a vector mulfix
            # applies the attention once sigmoided), everything else runs
            # fused. Each dense 18-MM main (~3.5us) separates every SE
            # producer from its consumer, hiding all drain/activation
            # latencies. SE deps: s(t) <- p(t) drains; q(t) <- relu of
            # s(t) AND s(t+1) (the dh=+1 row); r(t) <- q(t) drains.
            # mr = raw main (full tile), mh = raw main row-half (tiles 4/5
            # are halved so raw work is still available to fill the two
            # late-phase slots where no attention tile is sigmoided yet),
            # mf = fused main, x = mulfix. Every fused/x step sits AFTER
            # its r-step in program order (Tile deps follow program order).
            steps = [
                ("p", 0), ("p", 1), ("mr", 0, 0),
                ("s", 0), ("p", 2), ("mr", 1, 0),
                ("s", 1), ("mr", 2, 0), ("q", 0),
                ("p", 3), ("s", 2), ("mr", 3, 0),
                ("q", 1), ("r", 0), ("mf", 0, 1), ("x", 0),
                ("p", 4), ("s", 3), ("mh", 4, 0, 0),
                ("q", 2), ("r", 1), ("mf", 1, 1), ("x", 1),
                ("p", 5), ("s", 4), ("mh", 5, 0, 0),
                ("q", 3), ("r", 2), ("mf", 2, 1), ("x", 2),
                ("p", 6), ("s", 5), ("mh", 4, 0, 1),
                ("s", 6), ("q", 4), ("r", 3), ("mf", 3, 1), ("x", 3),
                ("mh", 5, 0, 1), ("q", 5),
                ("r", 4), ("mf", 4, 1), ("x", 4),
                ("q", 6), ("r", 5), ("mf", 5, 1), ("x", 5),
                ("r", 6), ("mf", 6, 0), ("mf", 6, 1),
            ]
            u1 = {}
            u2 = {}
            for step in steps:
                kind, t = step[0], step[1]
                if kind == "p":
                    u1[t] = conv1_pack(t)
                elif kind == "s":
                    conv1_sel(t, *u1[t])
                elif kind == "q":
                    u2[t] = conv2_pack(t)
                elif kind == "r":
                    conv2_sel(t, *u2[t])
                elif kind == "x":
                    mulfix(t, 0)
                elif kind == "mr":
                    main_raw(t, step[2])
                elif kind == "mh":
                    h = step[3]
                    main_raw(t, step[2], h * (RT // 2), (h + 1) * (RT // 2))
                else:
                    main_fused(t, step[2], split=(step in steps[-2:]))

    nc.compile()
    return nc


_NC = None


def make_in_maps(x, weight, A_w, se_w1, se_w2):
    import ml_dtypes

    bf16 = ml_dtypes.bfloat16
    x = np.asarray(x, dtype=np.float32)
    # pre-padded x: [B, 128, ci-block, guard + 56*58 + guard] with zero pad
    # columns (c=0, c=57) and guards; ci-blocks side by side per partition
    xpad = np.zeros((B, 128, 2, HWP + 2), dtype=bf16)
    xv = xpad[:, :, :, 1 : 1 + HWP].reshape(B, 128, 2, H, WP)
    xv[:, :, :, :, 1 : W + 1] = (
        x.reshape(B, 2, 128, H, W).transpose(0, 2, 1, 3, 4).astype(bf16)
    )
    xpad = xpad.reshape(B, 128, 2 * (HWP + 2))

    # fold A_w into the conv weight on host (f32), then transpose+cast;
    # layout [co-block, 128ci, ci-block, 9, 128co] so each co-block half is
    # one contiguous DMA
    wm = np.asarray(weight, dtype=np.float32) * np.asarray(A_w, dtype=np.float32)
    wmT = wm.transpose(1, 2, 3, 0).reshape(2, 128, 9, 2, 128).astype(bf16)
    wmodT = np.ascontiguousarray(
        wmT.transpose(3, 1, 0, 2, 4).reshape(2, 128, 2 * 9 * 128)
    )

    # SE weights pre-packed: w1 kw groups at 32-col strides per kh slice
    # (block 0 cols [0:288), block 1 [288:576)), w2 separate on 16 parts
    w1T = np.asarray(se_w1, dtype=np.float32).transpose(1, 2, 3, 0)  # [ci,kh,kw,16]
    w1P = np.zeros((2, 128, 3, 3, 32), dtype=bf16)
    w1P[:, :, :, :, :CMID] = w1T.reshape(2, 128, 3, 3, CMID).astype(bf16)
    sew1P = np.ascontiguousarray(
        w1P.reshape(2, 128, 288).transpose(1, 0, 2).reshape(128, 2 * 288)
    )
    w2P = np.zeros((CMID, 3, 3, 32), dtype=bf16)
    w2P[:, :, :, :CMID] = (
        np.asarray(se_w2, dtype=np.float32)[0].astype(bf16)[:, :, :, None]
    )
    sew2P = np.ascontiguousarray(w2P.reshape(CMID, 288))

    in_maps = [
        {
            "xpad": np.ascontiguousarray(xpad[b]),
            "wmodT": wmodT,
            "sew1P": sew1P,
            "sew2P": sew2P,
        }
        for b in range(B)
    ]
    return in_maps


def kernel(x, weight, A_w, se_w1, se_w2):
    global _NC
    if _NC is None:
        _NC = build()
    in_maps = make_in_maps(x, weight, A_w, se_w1, se_w2)
    res = run_bass_kernel_spmd(_NC, in_maps, list(range(N_CORES)))
    out = np.stack([res.results[b]["outp"] for b in range(B)], axis=0)
    # strip pad columns: [B,2,128,56*58] -> [B,256,56,56]
    out = out.reshape(B, 2, 128, H, WP)[:, :, :, :, 1 : W + 1].reshape(B, C, H, W)
    return np.ascontiguousarray(out)



# revision 39
# speedup vs baseline: 1.0096x; 1.0096x over previous
"""Trainium2 Bass kernel for: out = conv3x3(x, weight*A_w) * sigmoid(conv3x3(relu(conv3x3(x, se_w1)), se_w2))

Sharding: data-parallel over batch B=8 -> 8 NeuronCores (one image per core);
weights replicated. A_w is folded into the conv weight on the host (f32
multiply, then bf16 cast), so the device sees one fused weight tensor.

Per-core kernel (direct conv as implicit GEMM on the TensorEngine):
  - x stored column-padded [ci, 56, 58] bf16 in SBUF (pad cols zeroed,
    +1-element guards at both flat ends) so every 3x3 tap is a contiguous
    1-D shifted window (the matmul ISA requires single-free-dim operands).
  - row taps at the image top/bottom use clipped row ranges; the center tap
    is issued first per ci-block pass (full coverage, start=True), the
    clipped taps accumulate -> exact zero-padding semantics.
  - compute dtype bf16 (fp32 PSUM accumulate), rel-err vs fp32 ~3e-3.
  - x DMA is row-chunked (4 chunks, one 3-D trigger each covering both
    ci-blocks) and ordered ahead of the big main-conv weight on the sync
    queue; startup is DMA-bandwidth-bound, so the critical first deps (SE
    w1 + chunk A) get the gpsimd + sync queues while everything else
    queues behind in need-order.
  - SE kw-strip reduction m = s0(<<1) + s1 + s2(>>1) runs as a ScalarE
    strip copy + two shifted scalar_tensor_tensor adds on the DVE straight
    out of PSUM — the conv1 selector matmul is gone (the conv2 selector
    matmul remains: it also replicates the single attention channel to all
    128 partitions via a K=16 identity matmul).
  - schedule: main groups interleave with the SE chain from the VERY first
    step (an SE-only warm-up phase stalls the PE ~4us on PSUM-drain
    latencies): the c=0 groups for tiles 0-5 run RAW (conv drained to SBUF
    by ScalarE before any attention exists; a vector mulfix multiplies the
    attention in later and streams the tile out), tiles 4/5 split into row
    halves to fill the two late-phase slots with no sigmoided attention
    yet; every other main is fused (attention multiply straight out of
    PSUM + per-tile output DMA). Each dense 18-MM main (~3.5us) separates
    every SE producer from its consumer.
  - ScalarE activation tables (identity/relu/sigmoid) are pre-warmed with
    dummy activations during the DMA wait.
  - the last main group splits its mul+DMA into row halves on both queues
    to shorten the output tail.

Measured on HW: baseline-fused-schedule 86.6us -> this version ~84.0us.
Fixed costs: ~5.9us engine preamble before the measured window, ~5us
startup (DMA trigger latency ~1.4us + ~1.3MB critical input stream),
~11.4us tail after the last matmul (mul+DMA ~2 + framework teardown
barriers ~9, invariant to scheduling). PE floor (bf16 implicit GEMM,
1 moving column/cycle @2.4GHz): ~62us.

Rejected variants (measured slower): SE-branch-first scheduling (exposes
the serial PSUM-drain chain, 91us); 1-D F(2,3) Winograd main conv (24 MMs
of free 392 per 14-row tile instead of 36 of 464 per 16 rows, but the
7 extra [128,392] elementwise ops per group saturate Vector/Scalar, 101us);
spreading the startup loads across the scalar/gpsimd DMA queues in
parallel (queues split bandwidth evenly, starving whichever chunk is
needed next: +2..4us in three different arrangements); fp8 (quantization
noise ~5% exceeds the 2e-2 gate); packing kh into the SE stationary
(drain-strip count triples, saturating the DVE).
"""

import numpy as np

import concourse.bass as bass  # noqa: F401
import concourse.mybir as mybir
import concourse.tile as tile
from concourse import bacc
from concourse.bass_utils import run_bass_kernel_spmd
from concourse.masks import make_identity

B, C, H, W = 8, 256, 56, 56
HW = H * W
WP = W + 2                      # padded row width (c=0 left pad, c=57 right pad)
HWP = H * WP                    # 3248
CMID = 16
N_CORES = 8
RT = 8                          # output rows per PSUM tile
NT = H // RT                    # 7
F32 = mybir.dt.float32
BF16 = mybir.dt.bfloat16

# x DMA row-chunk boundaries: chunk A rows [0,10) serves SE tile 0,
# B rows [10,18) tile 1, C rows [18,34) tiles 2-3, D rows [34,56) tiles 4-6.
CH_A = 10
CH_B = 18
CH_C = 34

# center tap first within each ci-block pass
TAPS = [(0, 0)] + [
    (dh, dw) for dh in (-1, 0, 1) for dw in (-1, 0, 1) if (dh, dw) != (0, 0)
]


def _rows(r0, dh):
    """Clipped local row range [rl, rh) of a tile at base row r0 for row-tap dh."""
    return max(0, -dh - r0), min(RT, H - dh - r0)


def build():
    nc = bacc.Bacc("TRN2", target_bir_lowering=False, debug=False, num_devices=N_CORES)

    # x pre-padded on host: [128, ci-block * (1 + 56*58 + 1)] bf16 (both
    # ci-blocks side by side per partition) so each row-chunk loads with a
    # single 3-D DMA trigger; zero pad columns and flat-end guards baked in
    x_d = nc.dram_tensor("xpad", [128, 2 * (HWP + 2)], BF16, kind="ExternalInput").ap()
    # (weight * A_w) transposed on host, split by OUTPUT-channel block so
    # the c=0 weights (first raw main groups' dep) land first:
    # [co-block, 128ci, ci-block * 9 * 128co]
    wm_d = nc.dram_tensor(
        "wmodT", [2, 128, 2 * 9 * 128], BF16, kind="ExternalInput"
    ).ap()
    # SE weights pre-packed on host: w1 as one tensor (cols [0:288) block 0,
    # [288:576) block 1; kw groups at 32-col strides per kh), w2 separate
    w1_d = nc.dram_tensor("sew1P", [128, 2 * 288], BF16, kind="ExternalInput").ap()
    w2_d = nc.dram_tensor("sew2P", [CMID, 288], BF16, kind="ExternalInput").ap()
    # output in padded layout [ci-block, 128, 56*58]; host strips pad cols
    out_d = nc.dram_tensor("outp", [2, 128, HWP], F32, kind="ExternalOutput").ap()

    with tile.TileContext(nc) as tc:
        with (
            tc.tile_pool(name="sb", bufs=1) as sb,
            tc.tile_pool(name="ps", space="PSUM", bufs=2) as ps,
        ):
            asb = sb.tile([128, HWP], F32, name="asb")
            asig = sb.tile([CMID, RT * WP], F32, name="asig")
            osb = [sb.tile([128, HWP], F32, name=f"osb{c}") for c in range(2)]
            # +2: one guard element at each flat end (dw=+-1 at image corners)
            xsb = sb.tile([128, 2 * (HWP + 2)], BF16, name="xsb")
            xs = [xsb[:, i * (HWP + 2) : (i + 1) * (HWP + 2)] for i in range(2)]
            wmod = [sb.tile([128, 2 * 9 * 128], BF16, name=f"wmod{c}") for c in range(2)]
            mid = sb.tile([CMID, HWP + 2], BF16, name="mid")
            identE = sb.tile([96, CMID], BF16, name="identE")
            identTE = sb.tile([96, 128], BF16, name="identTE")
            u1pp = [sb.tile([96, RT * WP], BF16, name=f"u1pp{k}") for k in range(2)]
            u2pp = [sb.tile([96, RT * WP], BF16, name=f"u2pp{k}") for k in range(2)]
            wsb = sb.tile([128, 2 * 288], BF16, name="wsb")
            w1pack = [wsb[:, i * 288 : (i + 1) * 288] for i in range(2)]
            w2sb = sb.tile([CMID, 288], BF16, name="w2sb")
            w2pack = w2sb[:, :]

            # -------- loads --------
            # One trigger per transfer (each DMA_DIRECT2D costs ~0.6us of
            # engine time; data starts ~1.4us after the trigger). Queue
            # ORDER is the priority mechanism: parallel queues split DMA
            # bandwidth evenly, so the critical first-matmul deps (w1 +
            # chunk A, ~0.45MB total) get three queues to themselves while
            # the bulk weights queue up behind them (measured: wmod sharing
            # the early window starves chunk A and delays the first matmul
            # by 8us). Need-order per queue; the tiny w2 rides gpsimd.
            fA = 1 + CH_A * WP
            fB = 1 + CH_B * WP
            fC = 1 + CH_C * WP
            xsb_r = xsb.rearrange("p (i f) -> p i f", i=2)
            x_d_r = x_d.rearrange("p (i f) -> p i f", i=2)
            nc.gpsimd.dma_start(wsb, w1_d)
            nc.gpsimd.dma_start(w2sb, w2_d)
            nc.sync.dma_start(xsb_r[:, :, 0:fA], x_d_r[:, :, 0:fA])
            nc.sync.dma_start(xsb_r[:, :, fA:fB], x_d_r[:, :, fA:fB])
            nc.sync.dma_start(wmod[0], wm_d[0])
            nc.sync.dma_start(xsb_r[:, :, fB:fC], x_d_r[:, :, fB:fC])
            nc.sync.dma_start(xsb_r[:, :, fC : HWP + 2], x_d_r[:, :, fC : HWP + 2])
            nc.sync.dma_start(wmod[1], wm_d[1])

            def pad_memset(tl, np_):
                nc.vector.memset(tl[:np_, 0:2], 0.0)
                nc.vector.memset(tl[:np_, HWP : HWP + 2], 0.0)
                pads = tl[:np_, 1 + W + 1 : 1 + W + 1 + (H - 1) * WP].rearrange(
                    "p (h c) -> p h c", c=WP
                )
                nc.vector.memset(pads[:, :, 0:2], 0.0)

            # -------- prep (VectorE only, no PE) --------
            # pre-warm ScalarE activation tables (sigmoid/relu/identity)
            # during the DMA wait so the ~1.3us table loads don't stall the
            # SE dependency chain mid-kernel
            warm = sb.tile([1, 2], F32, name="warm")
            nc.vector.memset(warm, 0.0)
            for fn in (
                mybir.ActivationFunctionType.Identity,
                mybir.ActivationFunctionType.Relu,
                mybir.ActivationFunctionType.Sigmoid,
            ):
                nc.scalar.activation(warm[0:1, 0:1], warm[0:1, 1:2], fn)
            pad_memset(mid, CMID)
            for k in range(2):
                nc.vector.memset(u1pp[k], 0.0)
                nc.vector.memset(u2pp[k], 0.0)
            # identity selectors, one copy per 32-aligned strip (matmul
            # operands must share a 32-aligned partition base)
            nc.vector.memset(identE, 0.0)
            nc.vector.memset(identTE, 0.0)
            for g in range(3):
                make_identity(nc, identE[32 * g : 32 * g + CMID, :], nomemset=True)
                nc.vector.tensor_copy(
                    identTE[32 * g : 32 * g + CMID, :].rearrange(
                        "p (r c) -> p r c", c=CMID
                    ),
                    identE[32 * g : 32 * g + CMID, :]
                    .unsqueeze(1)
                    .broadcast_to([CMID, 8, CMID]),
                )

            mid_v = mid[:, 1 : 1 + HWP].rearrange("p (h c) -> p h c", c=WP)
            TFv = RT * WP
            wmod_v = [
                wmod[c].rearrange("p (i k co) -> p i k co", i=2, co=128)
                for c in range(2)
            ]

            # -------- conv group emitters --------
            # SE convs: the 3 kw taps are packed into the stationary columns
            # (48 = 3 kw x 16 ch). The kw-strip reduction m = s0(<<1) + s1 +
            # s2(>>1) runs as two shifted scalar_tensor_tensor adds on the
            # DVE straight out of PSUM (no selector matmul, no strip-drain
            # copies), keeping the PE and ScalarE out of the strip path.
            # Junk in pad columns only.
            ADD = mybir.AluOpType.add
            MUL = mybir.AluOpType.mult

            def strip_stt1(u, psrc):
                # the DVE allows only ONE PSUM operand per op: the middle
                # strip drains via ScalarE first (as before), then the stt
                # adds the +1-shifted s0 strip straight out of PSUM
                nc.scalar.activation(
                    u[32:48, :], psrc[32:48, :], mybir.ActivationFunctionType.Identity
                )
                nc.vector.scalar_tensor_tensor(
                    u[64:80, 1:TFv],
                    psrc[0:16, 0 : TFv - 1],
                    1.0,
                    u[32:48, 1:TFv],
                    MUL,
                    ADD,
                )

            def strip_stt2(u, psrc):
                # u[0:16] <- (s0<<1 + s1)  +  s2 shifted -1
                nc.vector.scalar_tensor_tensor(
                    u[0:16, 1 : TFv - 1],
                    u[64:80, 1 : TFv - 1],
                    1.0,
                    psrc[64:80, 2:TFv],
                    MUL,
                    ADD,
                )

            def conv1_pack(t):
                r0 = t * RT
                mps = ps.tile([96, TFv], F32, name="mps96", tag="pack", bufs=3)
                n_mm = 0
                for i in range(2):
                    for dh in (0, -1, 1):
                        kh = dh + 1
                        rl, rh = _rows(r0, dh)
                        n_mm += 1
                        nc.tensor.matmul(
                            mps[:, rl * WP : rh * WP],
                            w1pack[i][:, kh * 96 : (kh + 1) * 96],
                            xs[i][:, 1 + (r0 + rl + dh) * WP :][:128, : (rh - rl) * WP],
                            start=(n_mm == 1),
                            stop=(n_mm == 6),
                        )
                u = u1pp[t % 2]
                strip_stt1(u, mps)
                return u, mps

            def conv1_sel(t, u, mps):
                r0 = t * RT
                strip_stt2(u, mps)
                uv = u.rearrange("p (h c) -> p h c", c=WP)
                nc.scalar.activation(
                    mid_v[:, r0 : r0 + RT, 1 : W + 1],
                    uv[0:16, :, 1 : W + 1],
                    mybir.ActivationFunctionType.Relu,
                )

            def conv2_pack(t):
                r0 = t * RT
                ups = ps.tile([96, TFv], F32, name="u2ps", tag="pack", bufs=3)
                n_mm = 0
                for dh in (0, -1, 1):
                    kh = dh + 1
                    rl, rh = _rows(r0, dh)
                    n_mm += 1
                    nc.tensor.matmul(
                        ups[:, rl * WP : rh * WP],
                        w2pack[:, kh * 96 : (kh + 1) * 96],
                        mid[:, 1 + (r0 + rl + dh) * WP :][:CMID, : (rh - rl) * WP],
                        start=(n_mm == 1),
                        stop=(n_mm == 3),
                    )
                u = u2pp[t % 2]
                strip_stt1(u, ups)
                return u, ups

            def conv2_sel(t, u, ups):
                r0 = t * RT
                strip_stt2(u, ups)
                # sigmoid on the 16-partition strip, then the idle GpSimd
                # engine replicates the single attention channel to all 128
                # partitions (keeps the PE out of the attention path)
                nc.scalar.activation(
                    asig, u[0:16, :], mybir.ActivationFunctionType.Sigmoid
                )
                nc.gpsimd.partition_broadcast(
                    asb[:, r0 * WP : (r0 + RT) * WP], asig[0:1, :], channels=128
                )

            def main_mms(t, c, ra=0, rb=RT):
                r0 = t * RT
                yps = ps.tile([128, (rb - ra) * WP], F32, name="yps", tag="yps", bufs=3)
                n_mm = 0
                for i in range(2):
                    for dh, dw in TAPS:
                        k = (dh + 1) * 3 + (dw + 1)
                        rl = max(ra, -dh - r0)
                        rh = min(rb, H - dh - r0)
                        n_mm += 1
                        nc.tensor.matmul(
                            yps[:, (rl - ra) * WP : (rh - ra) * WP],
                            wmod_v[c][:, i, k, :],
                            xs[i][:, 1 + (r0 + rl + dh) * WP + dw :][:128, : (rh - rl) * WP],
                            start=(n_mm == 1),
                            stop=(n_mm == 18),
                        )
                return yps

            def main_fused(t, c, split=False):
                yps = main_mms(t, c)
                r0 = t * RT
                # the very last group splits its attention-mul + output DMA
                # into row halves on both queues so the final transfer is
                # half-size and overlaps the second mul (shorter tail)
                halves = ((0, RT // 2), (RT // 2, RT)) if split else ((0, RT),)
                for hi, (ra, rb) in enumerate(halves):
                    dst = osb[c][:, (r0 + ra) * WP : (r0 + rb) * WP]
                    amap = asb[:, (r0 + ra) * WP : (r0 + rb) * WP]
                    nc.vector.tensor_mul(dst, yps[:, ra * WP : rb * WP], amap)
                    q = nc.sync if (t + c + hi) % 2 == 0 else nc.scalar
                    q.dma_start(out_d[c][:, (r0 + ra) * WP : (r0 + rb) * WP], dst)

            def main_raw(t, c, ra=0, rb=RT):
                # conv only (attention map not yet available): drain the raw
                # conv result to SBUF on the scalar engine; mulfix() later
                # multiplies in the attention and streams the tile out
                yps = main_mms(t, c, ra, rb)
                r0 = t * RT
                nc.scalar.activation(
                    osb[c][:, (r0 + ra) * WP : (r0 + rb) * WP],
                    yps,
                    mybir.ActivationFunctionType.Identity,
                )

            def mulfix(t, c):
                r0 = t * RT
                dst = osb[c][:, r0 * WP : (r0 + RT) * WP]
                nc.vector.tensor_mul(dst, dst, asb[:, r0 * WP : (r0 + RT) * WP])
                q = nc.sync if (t + c) % 2 == 0 else nc.scalar
                q.dma_start(out_d[c][:, r0 * WP : (r0 + RT) * WP], dst)

            # -------- schedule ------------------------------------------
            # Main groups interleave with the SE chain from the very first
            # step (the SE chain alone stalls the PE ~4us on PSUM-drain
            # latencies): the six c=0 groups for tiles 0-5 run RAW (conv
            # drained to SBUF before any attention exists; a vector mulfix
            # applies the attention once sigmoided), everything else runs
            # fused. Each dense 18-MM main (~3.5us) separates every SE
            # producer from its consumer, hiding all drain/activation
            # latencies. SE deps: s(t) <- p(t) drains; q(t) <- relu of
            # s(t) AND s(t+1) (the dh=+1 row); r(t) <- q(t) drains.
            # mr = raw main (full tile), mh = raw main row-half (tiles 4/5
            # are halved so raw work is still available to fill the two
            # late-phase slots where no attention tile is sigmoided yet),
            # mf = fused main, x = mulfix. Every fused/x step sits AFTER
            # its r-step in program order (Tile deps follow program order).
            steps = [
                ("p", 0), ("p", 1), ("mr", 0, 0),
                ("s", 0), ("p", 2), ("mr", 1, 0),
                ("s", 1), ("mr", 2, 0), ("q", 0),
                ("p", 3), ("s", 2), ("mr", 3, 0),
                ("q", 1), ("r", 0), ("mf", 0, 1), ("x", 0),
                ("p", 4), ("s", 3), ("mh", 4, 0, 0),
                ("q", 2), ("r", 1), ("mf", 1, 1), ("x", 1),
                ("p", 5), ("s", 4), ("mh", 5, 0, 0),
                ("q", 3), ("r", 2), ("mf", 2, 1), ("x", 2),
                ("p", 6), ("s", 5), ("mh", 4, 0, 1),
                ("s", 6), ("q", 4), ("r", 3), ("mf", 3, 1), ("x", 3),
                ("mh", 5, 0, 1), ("q", 5),
                ("r", 4), ("mf", 4, 1), ("x", 4),
                ("q", 6), ("r", 5), ("mf", 5, 1), ("x", 5),
                ("r", 6), ("mf", 6, 0), ("mf", 6, 1),
            ]
            u1 = {}
            u2 = {}
            for step in steps:
                kind, t = step[0], step[1]
                if kind == "p":
                    u1[t] = conv1_pack(t)
                elif kind == "s":
                    conv1_sel(t, *u1[t])
                elif kind == "q":
                    u2[t] = conv2_pack(t)
                elif kind == "r":
                    conv2_sel(t, *u2[t])
                elif kind == "x":
                    mulfix(t, 0)
                elif kind == "mr":
                    main_raw(t, step[2])
                elif kind == "mh":
                    h = step[3]
                    main_raw(t, step[2], h * (RT // 2), (h + 1) * (RT // 2))
                else:
                    main_fused(t, step[2], split=(step == steps[-1]))

    nc.compile()
    return nc


_NC = None


def make_in_maps(x, weight, A_w, se_w1, se_w2):
    import ml_dtypes

    bf16 = ml_dtypes.bfloat16
    x = np.asarray(x, dtype=np.float32)
    # pre-padded x: [B, 128, ci-block, guard + 56*58 + guard] with zero pad
    # columns (c=0, c=57) and guards; ci-blocks side by side per partition
    xpad = np.zeros((B, 128, 2, HWP + 2), dtype=bf16)
    xv = xpad[:, :, :, 1 : 1 + HWP].reshape(B, 128, 2, H, WP)
    xv[:, :, :, :, 1 : W + 1] = (
        x.reshape(B, 2, 128, H, W).transpose(0, 2, 1, 3, 4).astype(bf16)
    )
    xpad = xpad.reshape(B, 128, 2 * (HWP + 2))

    # fold A_w into the conv weight on host (f32), then transpose+cast;
    # layout [co-block, 128ci, ci-block, 9, 128co] so each co-block half is
    # one contiguous DMA
    wm = np.asarray(weight, dtype=np.float32) * np.asarray(A_w, dtype=np.float32)
    wmT = wm.transpose(1, 2, 3, 0).reshape(2, 128, 9, 2, 128).astype(bf16)
    wmodT = np.ascontiguousarray(
        wmT.transpose(3, 1, 0, 2, 4).reshape(2, 128, 2 * 9 * 128)
    )

    # SE weights pre-packed: w1 kw groups at 32-col strides per kh slice
    # (block 0 cols [0:288), block 1 [288:576)), w2 separate on 16 parts
    w1T = np.asarray(se_w1, dtype=np.float32).transpose(1, 2, 3, 0)  # [ci,kh,kw,16]
    w1P = np.zeros((2, 128, 3, 3, 32), dtype=bf16)
    w1P[:, :, :, :, :CMID] = w1T.reshape(2, 128, 3, 3, CMID).astype(bf16)
    sew1P = np.ascontiguousarray(
        w1P.reshape(2, 128, 288).transpose(1, 0, 2).reshape(128, 2 * 288)
    )
    w2P = np.zeros((CMID, 3, 3, 32), dtype=bf16)
    w2P[:, :, :, :CMID] = (
        np.asarray(se_w2, dtype=np.float32)[0].astype(bf16)[:, :, :, None]
    )
    sew2P = np.ascontiguousarray(w2P.reshape(CMID, 288))

    in_maps = [
        {
            "xpad": np.ascontiguousarray(xpad[b]),
            "wmodT": wmodT,
            "sew1P": sew1P,
            "sew2P": sew2P,
        }
        for b in range(B)
    ]
    return in_maps


def kernel(x, weight, A_w, se_w1, se_w2):
    global _NC
    if _NC is None:
        _NC = build()
    in_maps = make_in_maps(x, weight, A_w, se_w1, se_w2)
    res = run_bass_kernel_spmd(_NC, in_maps, list(range(N_CORES)))
    out = np.stack([res.results[b]["outp"] for b in range(B)], axis=0)
    # strip pad columns: [B,2,128,56*58] -> [B,256,56,56]
    out = out.reshape(B, 2, 128, H, WP)[:, :, :, :, 1 : W + 1].reshape(B, C, H, W)
    return np.ascontiguousarray(out)



# revision 41
# speedup vs baseline: 1.0108x; 1.0012x over previous
"""Trainium2 Bass kernel for: out = conv3x3(x, weight*A_w) * sigmoid(conv3x3(relu(conv3x3(x, se_w1)), se_w2))

Sharding: data-parallel over batch B=8 -> 8 NeuronCores (one image per core);
weights replicated. A_w is folded into the conv weight on the host (f32
multiply, then bf16 cast), so the device sees one fused weight tensor.

Per-core kernel (direct conv as implicit GEMM on the TensorEngine):
  - x stored column-padded [ci, 56, 58] bf16 in SBUF (pad cols zeroed,
    +1-element guards at both flat ends) so every 3x3 tap is a contiguous
    1-D shifted window (the matmul ISA requires single-free-dim operands).
  - row taps at the image top/bottom use clipped row ranges; the center tap
    is issued first per ci-block pass (full coverage, start=True), the
    clipped taps accumulate -> exact zero-padding semantics.
  - compute dtype bf16 (fp32 PSUM accumulate), rel-err vs fp32 ~3e-3.
  - x DMA is row-chunked (4 chunks, one 3-D trigger each covering both
    ci-blocks) and ordered ahead of the big main-conv weight on the sync
    queue; startup is DMA-bandwidth-bound, so the critical first deps (SE
    w1 + chunk A) get the gpsimd + sync queues while everything else
    queues behind in need-order.
  - SE kw-strip reduction m = s0(<<1) + s1 + s2(>>1) runs as a ScalarE
    strip copy + two shifted scalar_tensor_tensor adds on the DVE straight
    out of PSUM — BOTH selector matmuls are gone: conv1 needs none, and
    conv2's 16->128 attention replication rides the otherwise-idle GpSimd
    engine (partition_broadcast after a 16-partition sigmoid).
  - schedule: main groups interleave with the SE chain from the VERY first
    step (an SE-only warm-up phase stalls the PE ~4us on PSUM-drain
    latencies): the c=0 groups for tiles 0-5 run RAW (conv drained to SBUF
    by ScalarE before any attention exists; a vector mulfix multiplies the
    attention in later and streams the tile out), tiles 4/5 split into row
    halves to fill the two late-phase slots with no sigmoided attention
    yet; every other main is fused (attention multiply straight out of
    PSUM + per-tile output DMA). Each dense 18-MM main (~3.5us) separates
    every SE producer from its consumer.
  - ScalarE activation tables (identity/relu/sigmoid) are pre-warmed with
    dummy activations during the DMA wait.
  - the last main group splits its mul+DMA into row halves on both queues
    to shorten the output tail.

Measured on HW: baseline-fused-schedule 86.6us -> this version 81.6-82.1us.
Fixed costs: ~5.9us engine preamble before the measured window, ~5us
startup (DMA trigger latency ~1.4us + ~1.3MB critical input stream),
~11.4us tail after the last matmul (mul+DMA ~2 + framework teardown
barriers ~9, invariant to scheduling). PE floor (bf16 implicit GEMM,
1 moving column/cycle @2.4GHz): ~61.5us; matmul span runs ~66us (the
residual ~3us is startup-window DMA/SBUF contention on the early packs
plus ~50-150ns of sem-wait + LDWEIGHTS exposure per accumulation group).

Rejected variants (measured slower): SE-branch-first scheduling (exposes
the serial PSUM-drain chain, 91us); 1-D F(2,3) Winograd main conv (24 MMs
of free 392 per 14-row tile instead of 36 of 464 per 16 rows, but the
7 extra [128,392] elementwise ops per group saturate Vector/Scalar, 101us);
spreading the startup loads across the scalar/gpsimd DMA queues in
parallel (queues split bandwidth evenly, starving whichever chunk is
needed next: +2..4us in three different arrangements); fp8 (quantization
noise ~5% exceeds the 2e-2 gate); packing kh into the SE stationary
(drain-strip count triples, saturating the DVE).
"""

import numpy as np

import concourse.bass as bass  # noqa: F401
import concourse.mybir as mybir
import concourse.tile as tile
from concourse import bacc
from concourse.bass_utils import run_bass_kernel_spmd
from concourse.masks import make_identity

B, C, H, W = 8, 256, 56, 56
HW = H * W
WP = W + 2                      # padded row width (c=0 left pad, c=57 right pad)
HWP = H * WP                    # 3248
CMID = 16
N_CORES = 8
RT = 8                          # output rows per PSUM tile
NT = H // RT                    # 7
F32 = mybir.dt.float32
BF16 = mybir.dt.bfloat16

# x DMA row-chunk boundaries: chunk A rows [0,10) serves SE tile 0,
# B rows [10,18) tile 1, C rows [18,34) tiles 2-3, D rows [34,56) tiles 4-6.
CH_A = 10
CH_B = 18
CH_C = 34

# center tap first within each ci-block pass
TAPS = [(0, 0)] + [
    (dh, dw) for dh in (-1, 0, 1) for dw in (-1, 0, 1) if (dh, dw) != (0, 0)
]


def _rows(r0, dh):
    """Clipped local row range [rl, rh) of a tile at base row r0 for row-tap dh."""
    return max(0, -dh - r0), min(RT, H - dh - r0)


def build():
    nc = bacc.Bacc("TRN2", target_bir_lowering=False, debug=False, num_devices=N_CORES)

    # x pre-padded on host: [128, ci-block * (1 + 56*58 + 1)] bf16 (both
    # ci-blocks side by side per partition) so each row-chunk loads with a
    # single 3-D DMA trigger; zero pad columns and flat-end guards baked in
    x_d = nc.dram_tensor("xpad", [128, 2 * (HWP + 2)], BF16, kind="ExternalInput").ap()
    # (weight * A_w) transposed on host, split by OUTPUT-channel block so
    # the c=0 weights (first raw main groups' dep) land first:
    # [co-block, 128ci, ci-block * 9 * 128co]
    wm_d = nc.dram_tensor(
        "wmodT", [2, 128, 2 * 9 * 128], BF16, kind="ExternalInput"
    ).ap()
    # SE weights pre-packed on host: w1 as one tensor (cols [0:288) block 0,
    # [288:576) block 1; kw groups at 32-col strides per kh), w2 separate
    w1_d = nc.dram_tensor("sew1P", [128, 2 * 288], BF16, kind="ExternalInput").ap()
    w2_d = nc.dram_tensor("sew2P", [CMID, 288], BF16, kind="ExternalInput").ap()
    # output in padded layout [ci-block, 128, 56*58]; host strips pad cols
    out_d = nc.dram_tensor("outp", [2, 128, HWP], F32, kind="ExternalOutput").ap()

    with tile.TileContext(nc) as tc:
        with (
            tc.tile_pool(name="sb", bufs=1) as sb,
            tc.tile_pool(name="ps", space="PSUM", bufs=2) as ps,
        ):
            asb = sb.tile([128, HWP], F32, name="asb")
            asig = sb.tile([CMID, RT * WP], F32, name="asig")
            osb = [sb.tile([128, HWP], F32, name=f"osb{c}") for c in range(2)]
            # +2: one guard element at each flat end (dw=+-1 at image corners)
            xsb = sb.tile([128, 2 * (HWP + 2)], BF16, name="xsb")
            xs = [xsb[:, i * (HWP + 2) : (i + 1) * (HWP + 2)] for i in range(2)]
            wmod = [sb.tile([128, 2 * 9 * 128], BF16, name=f"wmod{c}") for c in range(2)]
            mid = sb.tile([CMID, HWP + 2], BF16, name="mid")
            identE = sb.tile([96, CMID], BF16, name="identE")
            identTE = sb.tile([96, 128], BF16, name="identTE")
            u1pp = [sb.tile([96, RT * WP], BF16, name=f"u1pp{k}") for k in range(2)]
            u2pp = [sb.tile([96, RT * WP], BF16, name=f"u2pp{k}") for k in range(2)]
            wsb = sb.tile([128, 2 * 288], BF16, name="wsb")
            w1pack = [wsb[:, i * 288 : (i + 1) * 288] for i in range(2)]
            w2sb = sb.tile([CMID, 288], BF16, name="w2sb")
            w2pack = w2sb[:, :]

            # -------- loads --------
            # One trigger per transfer (each DMA_DIRECT2D costs ~0.6us of
            # engine time; data starts ~1.4us after the trigger). Queue
            # ORDER is the priority mechanism: parallel queues split DMA
            # bandwidth evenly, so the critical first-matmul deps (w1 +
            # chunk A, ~0.45MB total) get three queues to themselves while
            # the bulk weights queue up behind them (measured: wmod sharing
            # the early window starves chunk A and delays the first matmul
            # by 8us). Need-order per queue; the tiny w2 rides gpsimd.
            fA = 1 + CH_A * WP
            fB = 1 + CH_B * WP
            fC = 1 + CH_C * WP
            xsb_r = xsb.rearrange("p (i f) -> p i f", i=2)
            x_d_r = x_d.rearrange("p (i f) -> p i f", i=2)
            nc.gpsimd.dma_start(wsb, w1_d)
            nc.gpsimd.dma_start(w2sb, w2_d)
            nc.sync.dma_start(xsb_r[:, :, 0:fA], x_d_r[:, :, 0:fA])
            nc.sync.dma_start(xsb_r[:, :, fA:fB], x_d_r[:, :, fA:fB])
            nc.sync.dma_start(wmod[0], wm_d[0])
            nc.sync.dma_start(xsb_r[:, :, fB:fC], x_d_r[:, :, fB:fC])
            nc.sync.dma_start(xsb_r[:, :, fC : HWP + 2], x_d_r[:, :, fC : HWP + 2])
            nc.sync.dma_start(wmod[1], wm_d[1])

            def pad_memset(tl, np_):
                nc.vector.memset(tl[:np_, 0:2], 0.0)
                nc.vector.memset(tl[:np_, HWP : HWP + 2], 0.0)
                pads = tl[:np_, 1 + W + 1 : 1 + W + 1 + (H - 1) * WP].rearrange(
                    "p (h c) -> p h c", c=WP
                )
                nc.vector.memset(pads[:, :, 0:2], 0.0)

            # -------- prep (VectorE only, no PE) --------
            # pre-warm ScalarE activation tables (sigmoid/relu/identity)
            # during the DMA wait so the ~1.3us table loads don't stall the
            # SE dependency chain mid-kernel
            warm = sb.tile([1, 2], F32, name="warm")
            nc.vector.memset(warm, 0.0)
            for fn in (
                mybir.ActivationFunctionType.Identity,
                mybir.ActivationFunctionType.Relu,
                mybir.ActivationFunctionType.Sigmoid,
            ):
                nc.scalar.activation(warm[0:1, 0:1], warm[0:1, 1:2], fn)
            pad_memset(mid, CMID)
            for k in range(2):
                nc.vector.memset(u1pp[k], 0.0)
                nc.vector.memset(u2pp[k], 0.0)
            # identity selectors, one copy per 32-aligned strip (matmul
            # operands must share a 32-aligned partition base)
            nc.vector.memset(identE, 0.0)
            nc.vector.memset(identTE, 0.0)
            for g in range(3):
                make_identity(nc, identE[32 * g : 32 * g + CMID, :], nomemset=True)
                nc.vector.tensor_copy(
                    identTE[32 * g : 32 * g + CMID, :].rearrange(
                        "p (r c) -> p r c", c=CMID
                    ),
                    identE[32 * g : 32 * g + CMID, :]
                    .unsqueeze(1)
                    .broadcast_to([CMID, 8, CMID]),
                )

            mid_v = mid[:, 1 : 1 + HWP].rearrange("p (h c) -> p h c", c=WP)
            TFv = RT * WP
            wmod_v = [
                wmod[c].rearrange("p (i k co) -> p i k co", i=2, co=128)
                for c in range(2)
            ]

            # -------- conv group emitters --------
            # SE convs: the 3 kw taps are packed into the stationary columns
            # (48 = 3 kw x 16 ch). The kw-strip reduction m = s0(<<1) + s1 +
            # s2(>>1) runs as two shifted scalar_tensor_tensor adds on the
            # DVE straight out of PSUM (no selector matmul, no strip-drain
            # copies), keeping the PE and ScalarE out of the strip path.
            # Junk in pad columns only.
            ADD = mybir.AluOpType.add
            MUL = mybir.AluOpType.mult

            def strip_stt1(u, psrc):
                # the DVE allows only ONE PSUM operand per op: the middle
                # strip drains via ScalarE first (as before), then the stt
                # adds the +1-shifted s0 strip straight out of PSUM
                nc.scalar.activation(
                    u[32:48, :], psrc[32:48, :], mybir.ActivationFunctionType.Identity
                )
                nc.vector.scalar_tensor_tensor(
                    u[64:80, 1:TFv],
                    psrc[0:16, 0 : TFv - 1],
                    1.0,
                    u[32:48, 1:TFv],
                    MUL,
                    ADD,
                )

            def strip_stt2(u, psrc):
                # u[0:16] <- (s0<<1 + s1)  +  s2 shifted -1
                nc.vector.scalar_tensor_tensor(
                    u[0:16, 1 : TFv - 1],
                    u[64:80, 1 : TFv - 1],
                    1.0,
                    psrc[64:80, 2:TFv],
                    MUL,
                    ADD,
                )

            def conv1_pack(t):
                r0 = t * RT
                mps = ps.tile([96, TFv], F32, name="mps96", tag="pack", bufs=3)
                n_mm = 0
                for i in range(2):
                    for dh in (0, -1, 1):
                        kh = dh + 1
                        rl, rh = _rows(r0, dh)
                        n_mm += 1
                        nc.tensor.matmul(
                            mps[:, rl * WP : rh * WP],
                            w1pack[i][:, kh * 96 : (kh + 1) * 96],
                            xs[i][:, 1 + (r0 + rl + dh) * WP :][:128, : (rh - rl) * WP],
                            start=(n_mm == 1),
                            stop=(n_mm == 6),
                        )
                u = u1pp[t % 2]
                strip_stt1(u, mps)
                return u, mps

            def conv1_sel(t, u, mps):
                r0 = t * RT
                strip_stt2(u, mps)
                uv = u.rearrange("p (h c) -> p h c", c=WP)
                nc.scalar.activation(
                    mid_v[:, r0 : r0 + RT, 1 : W + 1],
                    uv[0:16, :, 1 : W + 1],
                    mybir.ActivationFunctionType.Relu,
                )

            def conv2_pack(t):
                r0 = t * RT
                ups = ps.tile([96, TFv], F32, name="u2ps", tag="pack", bufs=3)
                n_mm = 0
                for dh in (0, -1, 1):
                    kh = dh + 1
                    rl, rh = _rows(r0, dh)
                    n_mm += 1
                    nc.tensor.matmul(
                        ups[:, rl * WP : rh * WP],
                        w2pack[:, kh * 96 : (kh + 1) * 96],
                        mid[:, 1 + (r0 + rl + dh) * WP :][:CMID, : (rh - rl) * WP],
                        start=(n_mm == 1),
                        stop=(n_mm == 3),
                    )
                u = u2pp[t % 2]
                strip_stt1(u, ups)
                return u, ups

            def conv2_sel(t, u, ups):
                r0 = t * RT
                strip_stt2(u, ups)
                # sigmoid on the 16-partition strip, then the idle GpSimd
                # engine replicates the single attention channel to all 128
                # partitions (keeps the PE out of the attention path)
                nc.scalar.activation(
                    asig, u[0:16, :], mybir.ActivationFunctionType.Sigmoid
                )
                nc.gpsimd.partition_broadcast(
                    asb[:, r0 * WP : (r0 + RT) * WP], asig[0:1, :], channels=128
                )

            def main_mms(t, c, ra=0, rb=RT):
                r0 = t * RT
                yps = ps.tile([128, (rb - ra) * WP], F32, name="yps", tag="yps", bufs=3)
                n_mm = 0
                for i in range(2):
                    for dh, dw in TAPS:
                        k = (dh + 1) * 3 + (dw + 1)
                        rl = max(ra, -dh - r0)
                        rh = min(rb, H - dh - r0)
                        n_mm += 1
                        nc.tensor.matmul(
                            yps[:, (rl - ra) * WP : (rh - ra) * WP],
                            wmod_v[c][:, i, k, :],
                            xs[i][:, 1 + (r0 + rl + dh) * WP + dw :][:128, : (rh - rl) * WP],
                            start=(n_mm == 1),
                            stop=(n_mm == 18),
                        )
                return yps

            def main_fused(t, c, split=False):
                yps = main_mms(t, c)
                r0 = t * RT
                # the very last group splits its attention-mul + output DMA
                # into row halves on both queues so the final transfer is
                # half-size and overlaps the second mul (shorter tail)
                halves = ((0, RT // 2), (RT // 2, RT)) if split else ((0, RT),)
                for hi, (ra, rb) in enumerate(halves):
                    dst = osb[c][:, (r0 + ra) * WP : (r0 + rb) * WP]
                    amap = asb[:, (r0 + ra) * WP : (r0 + rb) * WP]
                    nc.vector.tensor_mul(dst, yps[:, ra * WP : rb * WP], amap)
                    q = nc.sync if (t + c + hi) % 2 == 0 else nc.scalar
                    q.dma_start(out_d[c][:, (r0 + ra) * WP : (r0 + rb) * WP], dst)

            def main_raw(t, c, ra=0, rb=RT):
                # conv only (attention map not yet available): drain the raw
                # conv result to SBUF on the scalar engine; mulfix() later
                # multiplies in the attention and streams the tile out
                yps = main_mms(t, c, ra, rb)
                r0 = t * RT
                nc.scalar.activation(
                    osb[c][:, (r0 + ra) * WP : (r0 + rb) * WP],
                    yps,
                    mybir.ActivationFunctionType.Identity,
                )

            def mulfix(t, c):
                r0 = t * RT
                dst = osb[c][:, r0 * WP : (r0 + RT) * WP]
                nc.vector.tensor_mul(dst, dst, asb[:, r0 * WP : (r0 + RT) * WP])
                q = nc.sync if (t + c) % 2 == 0 else nc.scalar
                q.dma_start(out_d[c][:, r0 * WP : (r0 + RT) * WP], dst)

            # -------- schedule ------------------------------------------
            # Main groups interleave with the SE chain from the very first
            # step (the SE chain alone stalls the PE ~4us on PSUM-drain
            # latencies): the six c=0 groups for tiles 0-5 run RAW (conv
            # drained to SBUF before any attention exists; a vector mulfix
            # applies the attention once sigmoided), everything else runs
            # fused. Each dense 18-MM main (~3.5us) separates every SE
            # producer from its consumer, hiding all drain/activation
            # latencies. SE deps: s(t) <- p(t) drains; q(t) <- relu of
            # s(t) AND s(t+1) (the dh=+1 row); r(t) <- q(t) drains.
            # mr = raw main (full tile), mh = raw main row-half (tiles 4/5
            # are halved so raw work is still available to fill the two
            # late-phase slots where no attention tile is sigmoided yet),
            # mf = fused main, x = mulfix. Every fused/x step sits AFTER
            # its r-step in program order (Tile deps follow program order).
            steps = [
                ("p", 0), ("p", 1), ("mr", 0, 0),
                ("s", 0), ("p", 2), ("mr", 1, 0),
                ("s", 1), ("mr", 2, 0), ("q", 0),
                ("p", 3), ("s", 2), ("mr", 3, 0),
                ("q", 1), ("r", 0), ("mf", 0, 1), ("x", 0),
                ("p", 4), ("s", 3), ("mh", 4, 0, 0),
                ("q", 2), ("r", 1), ("mf", 1, 1), ("x", 1),
                ("p", 5), ("s", 4), ("mh", 5, 0, 0),
                ("q", 3), ("r", 2), ("mf", 2, 1), ("x", 2),
                ("p", 6), ("s", 5), ("mh", 4, 0, 1),
                ("s", 6), ("q", 4), ("r", 3), ("mf", 3, 1), ("x", 3),
                ("mh", 5, 0, 1), ("q", 5),
                ("r", 4), ("mf", 4, 1), ("x", 4),
                ("q", 6), ("r", 5), ("mf", 5, 1), ("x", 5),
                ("r", 6), ("mf", 6, 0), ("mf", 6, 1),
            ]
            u1 = {}
            u2 = {}
            for step in steps:
                kind, t = step[0], step[1]
                if kind == "p":
                    u1[t] = conv1_pack(t)
                elif kind == "s":
                    conv1_sel(t, *u1[t])
                elif kind == "q":
                    u2[t] = conv2_pack(t)
                elif kind == "r":
                    conv2_sel(t, *u2[t])
                elif kind == "x":
                    mulfix(t, 0)
                elif kind == "mr":
                    main_raw(t, step[2])
                elif kind == "mh":
                    h = step[3]
                    main_raw(t, step[2], h * (RT // 2), (h + 1) * (RT // 2))
                else:
                    main_fused(t, step[2], split=(step == steps[-1]))

    nc.compile()
    return nc


_NC = None


def make_in_maps(x, weight, A_w, se_w1, se_w2):
    import ml_dtypes

    bf16 = ml_dtypes.bfloat16
    x = np.asarray(x, dtype=np.float32)
    # pre-padded x: [B, 128, ci-block, guard + 56*58 + guard] with zero pad
    # columns (c=0, c=57) and guards; ci-blocks side by side per partition
    xpad = np.zeros((B, 128, 2, HWP + 2), dtype=bf16)
    xv = xpad[:, :, :, 1 : 1 + HWP].reshape(B, 128, 2, H, WP)
    xv[:, :, :, :, 1 : W + 1] = (
        x.reshape(B, 2, 128, H, W).transpose(0, 2, 1, 3, 4).astype(bf16)
    )
    xpad = xpad.reshape(B, 128, 2 * (HWP + 2))

    # fold A_w into the conv weight on host (f32), then transpose+cast;
    # layout [co-block, 128ci, ci-block, 9, 128co] so each co-block half is
    # one contiguous DMA
    wm = np.asarray(weight, dtype=np.float32) * np.asarray(A_w, dtype=np.float32)
    wmT = wm.transpose(1, 2, 3, 0).reshape(2, 128, 9, 2, 128).astype(bf16)
    wmodT = np.ascontiguousarray(
        wmT.transpose(3, 1, 0, 2, 4).reshape(2, 128, 2 * 9 * 128)
    )

    # SE weights pre-packed: w1 kw groups at 32-col strides per kh slice
    # (block 0 cols [0:288), block 1 [288:576)), w2 separate on 16 parts
    w1T = np.asarray(se_w1, dtype=np.float32).transpose(1, 2, 3, 0)  # [ci,kh,kw,16]
    w1P = np.zeros((2, 128, 3, 3, 32), dtype=bf16)
    w1P[:, :, :, :, :CMID] = w1T.reshape(2, 128, 3, 3, CMID).astype(bf16)
    sew1P = np.ascontiguousarray(
        w1P.reshape(2, 128, 288).transpose(1, 0, 2).reshape(128, 2 * 288)
    )
    w2P = np.zeros((CMID, 3, 3, 32), dtype=bf16)
    w2P[:, :, :, :CMID] = (
        np.asarray(se_w2, dtype=np.float32)[0].astype(bf16)[:, :, :, None]
    )
    sew2P = np.ascontiguousarray(w2P.reshape(CMID, 288))

    in_maps = [
        {
            "xpad": np.ascontiguousarray(xpad[b]),
            "wmodT": wmodT,
            "sew1P": sew1P,
            "sew2P": sew2P,
        }
        for b in range(B)
    ]
    return in_maps


def kernel(x, weight, A_w, se_w1, se_w2):
    global _NC
    if _NC is None:
        _NC = build()
    in_maps = make_in_maps(x, weight, A_w, se_w1, se_w2)
    res = run_bass_kernel_spmd(_NC, in_maps, list(range(N_CORES)))
    out = np.stack([res.results[b]["outp"] for b in range(B)], axis=0)
    # strip pad columns: [B,2,128,56*58] -> [B,256,56,56]
    out = out.reshape(B, 2, 128, H, WP)[:, :, :, :, 1 : W + 1].reshape(B, C, H, W)
    return np.ascontiguousarray(out)



# revision 42
# speedup vs baseline: 1.0232x; 1.0123x over previous
"""Trainium2 Bass kernel for: out = conv3x3(x, weight*A_w) * sigmoid(conv3x3(relu(conv3x3(x, se_w1)), se_w2))

Sharding: data-parallel over batch B=8 -> 8 NeuronCores (one image per core);
weights replicated. A_w is folded into the conv weight on the host (f32
multiply, then bf16 cast), so the device sees one fused weight tensor.

Per-core kernel (direct conv as implicit GEMM on the TensorEngine):
  - x stored column-padded [ci, 56, 58] bf16 in SBUF (pad cols zeroed,
    +1-element guards at both flat ends) so every 3x3 tap is a contiguous
    1-D shifted window (the matmul ISA requires single-free-dim operands).
  - row taps at the image top/bottom use clipped row ranges; the center tap
    is issued first per ci-block pass (full coverage, start=True), the
    clipped taps accumulate -> exact zero-padding semantics.
  - compute dtype bf16 (fp32 PSUM accumulate), rel-err vs fp32 ~3e-3.
  - x DMA is row-chunked (4 chunks, one 3-D trigger each covering both
    ci-blocks) and ordered ahead of the big main-conv weight on the sync
    queue; startup is DMA-bandwidth-bound, so the critical first deps (SE
    w1 + chunk A) get the gpsimd + sync queues while everything else
    queues behind in need-order.
  - SE kw-strip reduction m = s0(<<1) + s1 + s2(>>1) runs as a ScalarE
    strip copy + two shifted scalar_tensor_tensor adds on the DVE straight
    out of PSUM — BOTH selector matmuls are gone: conv1 needs none, and
    conv2's 16->128 attention replication rides the otherwise-idle GpSimd
    engine (partition_broadcast after a 16-partition sigmoid).
  - schedule: main groups interleave with the SE chain from the VERY first
    step (an SE-only warm-up phase stalls the PE ~4us on PSUM-drain
    latencies): the c=0 groups for tiles 0-5 run RAW (conv drained to SBUF
    by ScalarE before any attention exists; a vector mulfix multiplies the
    attention in later and streams the tile out), tiles 4/5 split into row
    halves to fill the two late-phase slots with no sigmoided attention
    yet; every other main is fused (attention multiply straight out of
    PSUM + per-tile output DMA). Each dense 18-MM main (~3.5us) separates
    every SE producer from its consumer.
  - ScalarE activation tables (identity/relu/sigmoid) are pre-warmed with
    dummy activations during the DMA wait.
  - the last main group splits its mul+DMA into row halves on both queues
    to shorten the output tail.

Measured on HW: baseline-fused-schedule 86.6us -> this version 81.6-82.1us.
Fixed costs: ~5.9us engine preamble before the measured window, ~5us
startup (DMA trigger latency ~1.4us + ~1.3MB critical input stream),
~11.4us tail after the last matmul (mul+DMA ~2 + framework teardown
barriers ~9, invariant to scheduling). PE floor (bf16 implicit GEMM,
1 moving column/cycle @2.4GHz): ~61.5us; matmul span runs ~66us (the
residual ~3us is startup-window DMA/SBUF contention on the early packs
plus ~50-150ns of sem-wait + LDWEIGHTS exposure per accumulation group).

Rejected variants (measured slower): SE-branch-first scheduling (exposes
the serial PSUM-drain chain, 91us); 1-D F(2,3) Winograd main conv (24 MMs
of free 392 per 14-row tile instead of 36 of 464 per 16 rows, but the
7 extra [128,392] elementwise ops per group saturate Vector/Scalar, 101us);
spreading the startup loads across the scalar/gpsimd DMA queues in
parallel (queues split bandwidth evenly, starving whichever chunk is
needed next: +2..4us in three different arrangements); fp8 (quantization
noise ~5% exceeds the 2e-2 gate); packing kh into the SE stationary
(drain-strip count triples, saturating the DVE).
"""

import numpy as np

import concourse.bass as bass  # noqa: F401
import concourse.mybir as mybir
import concourse.tile as tile
from concourse import bacc
from concourse.bass_utils import run_bass_kernel_spmd
from concourse.masks import make_identity

B, C, H, W = 8, 256, 56, 56
HW = H * W
WP = W + 2                      # padded row width (c=0 left pad, c=57 right pad)
HWP = H * WP                    # 3248
CMID = 16
N_CORES = 8
RT = 8                          # output rows per PSUM tile
NT = H // RT                    # 7
F32 = mybir.dt.float32
BF16 = mybir.dt.bfloat16

# x DMA row-chunk boundaries: chunk A rows [0,10) serves SE tile 0,
# B rows [10,18) tile 1, C rows [18,34) tiles 2-3, D rows [34,56) tiles 4-6.
CH_A = 10
CH_B = 18
CH_C = 34

# center tap first within each ci-block pass
TAPS = [(0, 0)] + [
    (dh, dw) for dh in (-1, 0, 1) for dw in (-1, 0, 1) if (dh, dw) != (0, 0)
]


def _rows(r0, dh):
    """Clipped local row range [rl, rh) of a tile at base row r0 for row-tap dh."""
    return max(0, -dh - r0), min(RT, H - dh - r0)


def build():
    nc = bacc.Bacc("TRN2", target_bir_lowering=False, debug=False, num_devices=N_CORES)

    # x pre-padded on host: [128, ci-block * (1 + 56*58 + 1)] bf16 (both
    # ci-blocks side by side per partition) so each row-chunk loads with a
    # single 3-D DMA trigger; zero pad columns and flat-end guards baked in
    x_d = nc.dram_tensor("xpad", [128, 2 * (HWP + 2)], BF16, kind="ExternalInput").ap()
    # (weight * A_w) transposed on host, split by OUTPUT-channel block so
    # the c=0 weights (first raw main groups' dep) land first:
    # [co-block, 128ci, ci-block * 9 * 128co]
    wm_d = nc.dram_tensor(
        "wmodT", [2, 128, 2 * 9 * 128], BF16, kind="ExternalInput"
    ).ap()
    # SE weights pre-packed on host: w1 as one tensor (cols [0:288) block 0,
    # [288:576) block 1; kw groups at 32-col strides per kh), w2 separate
    w1_d = nc.dram_tensor("sew1P", [128, 2 * 288], BF16, kind="ExternalInput").ap()
    w2_d = nc.dram_tensor("sew2P", [CMID, 288], BF16, kind="ExternalInput").ap()
    # output in padded layout [ci-block, 128, 56*58]; host strips pad cols
    out_d = nc.dram_tensor("outp", [2, 128, HWP], F32, kind="ExternalOutput").ap()

    with tile.TileContext(nc) as tc:
        with (
            tc.tile_pool(name="sb", bufs=1) as sb,
            tc.tile_pool(name="ps", space="PSUM", bufs=2) as ps,
        ):
            asb = sb.tile([128, HWP], F32, name="asb")
            asig = sb.tile([CMID, RT * WP], F32, name="asig")
            osb = [sb.tile([128, HWP], F32, name=f"osb{c}") for c in range(2)]
            # +2: one guard element at each flat end (dw=+-1 at image corners)
            xsb = sb.tile([128, 2 * (HWP + 2)], BF16, name="xsb")
            xs = [xsb[:, i * (HWP + 2) : (i + 1) * (HWP + 2)] for i in range(2)]
            wmod = [sb.tile([128, 2 * 9 * 128], BF16, name=f"wmod{c}") for c in range(2)]
            mid = sb.tile([CMID, HWP + 2], BF16, name="mid")
            identE = sb.tile([96, CMID], BF16, name="identE")
            identTE = sb.tile([96, 128], BF16, name="identTE")
            u1pp = [sb.tile([96, RT * WP], BF16, name=f"u1pp{k}") for k in range(2)]
            u2pp = [sb.tile([96, RT * WP], BF16, name=f"u2pp{k}") for k in range(2)]
            wsb = sb.tile([128, 2 * 288], BF16, name="wsb")
            w1pack = [wsb[:, i * 288 : (i + 1) * 288] for i in range(2)]
            w2sb = sb.tile([CMID, 288], BF16, name="w2sb")
            w2pack = w2sb[:, :]

            # -------- loads --------
            # One trigger per transfer (each DMA_DIRECT2D costs ~0.6us of
            # engine time; data starts ~1.4us after the trigger). Queue
            # ORDER is the priority mechanism: parallel queues split DMA
            # bandwidth evenly, so the critical first-matmul deps (w1 +
            # chunk A, ~0.45MB total) get three queues to themselves while
            # the bulk weights queue up behind them (measured: wmod sharing
            # the early window starves chunk A and delays the first matmul
            # by 8us). Need-order per queue; the tiny w2 rides gpsimd.
            fA = 1 + CH_A * WP
            fB = 1 + CH_B * WP
            fC = 1 + CH_C * WP
            xsb_r = xsb.rearrange("p (i f) -> p i f", i=2)
            x_d_r = x_d.rearrange("p (i f) -> p i f", i=2)
            # w1 and chunk A split by ci-block: the first pack's i=0 matmuls
            # depend only on block 0 of each, so finer DMA semaphore
            # granularity starts the PE ~0.5us earlier
            nc.gpsimd.dma_start(wsb[:, 0:288], w1_d[:, 0:288])
            nc.gpsimd.dma_start(wsb[:, 288:576], w1_d[:, 288:576])
            nc.gpsimd.dma_start(w2sb, w2_d)
            nc.sync.dma_start(xsb[:, 0:fA], x_d[:, 0:fA])
            nc.sync.dma_start(
                xsb[:, HWP + 2 : HWP + 2 + fA], x_d[:, HWP + 2 : HWP + 2 + fA]
            )
            nc.sync.dma_start(xsb_r[:, :, fA:fB], x_d_r[:, :, fA:fB])
            nc.sync.dma_start(wmod[0], wm_d[0])
            nc.sync.dma_start(xsb_r[:, :, fB:fC], x_d_r[:, :, fB:fC])
            nc.sync.dma_start(xsb_r[:, :, fC : HWP + 2], x_d_r[:, :, fC : HWP + 2])
            nc.sync.dma_start(wmod[1], wm_d[1])

            def pad_memset(tl, np_):
                nc.vector.memset(tl[:np_, 0:2], 0.0)
                nc.vector.memset(tl[:np_, HWP : HWP + 2], 0.0)
                pads = tl[:np_, 1 + W + 1 : 1 + W + 1 + (H - 1) * WP].rearrange(
                    "p (h c) -> p h c", c=WP
                )
                nc.vector.memset(pads[:, :, 0:2], 0.0)

            # -------- prep (VectorE only, no PE) --------
            # pre-warm ScalarE activation tables (sigmoid/relu/identity)
            # during the DMA wait so the ~1.3us table loads don't stall the
            # SE dependency chain mid-kernel
            warm = sb.tile([1, 2], F32, name="warm")
            nc.vector.memset(warm, 0.0)
            for fn in (
                mybir.ActivationFunctionType.Identity,
                mybir.ActivationFunctionType.Relu,
                mybir.ActivationFunctionType.Sigmoid,
            ):
                nc.scalar.activation(warm[0:1, 0:1], warm[0:1, 1:2], fn)
            pad_memset(mid, CMID)
            for k in range(2):
                nc.vector.memset(u1pp[k], 0.0)
                nc.vector.memset(u2pp[k], 0.0)
            # identity selectors, one copy per 32-aligned strip (matmul
            # operands must share a 32-aligned partition base)
            nc.vector.memset(identE, 0.0)
            nc.vector.memset(identTE, 0.0)
            for g in range(3):
                make_identity(nc, identE[32 * g : 32 * g + CMID, :], nomemset=True)
                nc.vector.tensor_copy(
                    identTE[32 * g : 32 * g + CMID, :].rearrange(
                        "p (r c) -> p r c", c=CMID
                    ),
                    identE[32 * g : 32 * g + CMID, :]
                    .unsqueeze(1)
                    .broadcast_to([CMID, 8, CMID]),
                )

            mid_v = mid[:, 1 : 1 + HWP].rearrange("p (h c) -> p h c", c=WP)
            TFv = RT * WP
            wmod_v = [
                wmod[c].rearrange("p (i k co) -> p i k co", i=2, co=128)
                for c in range(2)
            ]

            # -------- conv group emitters --------
            # SE convs: the 3 kw taps are packed into the stationary columns
            # (48 = 3 kw x 16 ch). The kw-strip reduction m = s0(<<1) + s1 +
            # s2(>>1) runs as two shifted scalar_tensor_tensor adds on the
            # DVE straight out of PSUM (no selector matmul, no strip-drain
            # copies), keeping the PE and ScalarE out of the strip path.
            # Junk in pad columns only.
            ADD = mybir.AluOpType.add
            MUL = mybir.AluOpType.mult

            def strip_stt1(u, psrc):
                # the DVE allows only ONE PSUM operand per op: the middle
                # strip drains via ScalarE first (as before), then the stt
                # adds the +1-shifted s0 strip straight out of PSUM
                nc.scalar.activation(
                    u[32:48, :], psrc[32:48, :], mybir.ActivationFunctionType.Identity
                )
                nc.vector.scalar_tensor_tensor(
                    u[64:80, 1:TFv],
                    psrc[0:16, 0 : TFv - 1],
                    1.0,
                    u[32:48, 1:TFv],
                    MUL,
                    ADD,
                )

            def strip_stt2(u, psrc):
                # u[0:16] <- (s0<<1 + s1)  +  s2 shifted -1
                nc.vector.scalar_tensor_tensor(
                    u[0:16, 1 : TFv - 1],
                    u[64:80, 1 : TFv - 1],
                    1.0,
                    psrc[64:80, 2:TFv],
                    MUL,
                    ADD,
                )

            def conv1_pack(t):
                r0 = t * RT
                mps = ps.tile([96, TFv], F32, name="mps96", tag="pack", bufs=3)
                n_mm = 0
                for i in range(2):
                    for dh in (0, -1, 1):
                        kh = dh + 1
                        rl, rh = _rows(r0, dh)
                        n_mm += 1
                        nc.tensor.matmul(
                            mps[:, rl * WP : rh * WP],
                            w1pack[i][:, kh * 96 : (kh + 1) * 96],
                            xs[i][:, 1 + (r0 + rl + dh) * WP :][:128, : (rh - rl) * WP],
                            start=(n_mm == 1),
                            stop=(n_mm == 6),
                        )
                u = u1pp[t % 2]
                strip_stt1(u, mps)
                return u, mps

            def conv1_sel(t, u, mps):
                r0 = t * RT
                strip_stt2(u, mps)
                uv = u.rearrange("p (h c) -> p h c", c=WP)
                nc.scalar.activation(
                    mid_v[:, r0 : r0 + RT, 1 : W + 1],
                    uv[0:16, :, 1 : W + 1],
                    mybir.ActivationFunctionType.Relu,
                )

            def conv2_pack(t):
                r0 = t * RT
                ups = ps.tile([96, TFv], F32, name="u2ps", tag="pack", bufs=3)
                n_mm = 0
                for dh in (0, -1, 1):
                    kh = dh + 1
                    rl, rh = _rows(r0, dh)
                    n_mm += 1
                    nc.tensor.matmul(
                        ups[:, rl * WP : rh * WP],
                        w2pack[:, kh * 96 : (kh + 1) * 96],
                        mid[:, 1 + (r0 + rl + dh) * WP :][:CMID, : (rh - rl) * WP],
                        start=(n_mm == 1),
                        stop=(n_mm == 3),
                    )
                u = u2pp[t % 2]
                strip_stt1(u, ups)
                return u, ups

            def conv2_sel(t, u, ups):
                r0 = t * RT
                strip_stt2(u, ups)
                # sigmoid on the 16-partition strip, then the idle GpSimd
                # engine replicates the single attention channel to all 128
                # partitions (keeps the PE out of the attention path)
                nc.scalar.activation(
                    asig, u[0:16, :], mybir.ActivationFunctionType.Sigmoid
                )
                nc.gpsimd.partition_broadcast(
                    asb[:, r0 * WP : (r0 + RT) * WP], asig[0:1, :], channels=128
                )

            def main_mms(t, c, ra=0, rb=RT):
                r0 = t * RT
                yps = ps.tile([128, (rb - ra) * WP], F32, name="yps", tag="yps", bufs=3)
                n_mm = 0
                for i in range(2):
                    for dh, dw in TAPS:
                        k = (dh + 1) * 3 + (dw + 1)
                        rl = max(ra, -dh - r0)
                        rh = min(rb, H - dh - r0)
                        n_mm += 1
                        nc.tensor.matmul(
                            yps[:, (rl - ra) * WP : (rh - ra) * WP],
                            wmod_v[c][:, i, k, :],
                            xs[i][:, 1 + (r0 + rl + dh) * WP + dw :][:128, : (rh - rl) * WP],
                            start=(n_mm == 1),
                            stop=(n_mm == 18),
                        )
                return yps

            def main_fused(t, c, split=False):
                yps = main_mms(t, c)
                r0 = t * RT
                # the very last group splits its attention-mul + output DMA
                # into row halves on both queues so the final transfer is
                # half-size and overlaps the second mul (shorter tail)
                halves = ((0, RT // 2), (RT // 2, RT)) if split else ((0, RT),)
                for hi, (ra, rb) in enumerate(halves):
                    dst = osb[c][:, (r0 + ra) * WP : (r0 + rb) * WP]
                    amap = asb[:, (r0 + ra) * WP : (r0 + rb) * WP]
                    nc.vector.tensor_mul(dst, yps[:, ra * WP : rb * WP], amap)
                    q = nc.sync if (t + c + hi) % 2 == 0 else nc.scalar
                    q.dma_start(out_d[c][:, (r0 + ra) * WP : (r0 + rb) * WP], dst)

            def main_raw(t, c, ra=0, rb=RT):
                # conv only (attention map not yet available): drain the raw
                # conv result to SBUF on the scalar engine; mulfix() later
                # multiplies in the attention and streams the tile out
                yps = main_mms(t, c, ra, rb)
                r0 = t * RT
                nc.scalar.activation(
                    osb[c][:, (r0 + ra) * WP : (r0 + rb) * WP],
                    yps,
                    mybir.ActivationFunctionType.Identity,
                )

            def mulfix(t, c):
                r0 = t * RT
                dst = osb[c][:, r0 * WP : (r0 + RT) * WP]
                nc.vector.tensor_mul(dst, dst, asb[:, r0 * WP : (r0 + RT) * WP])
                q = nc.sync if (t + c) % 2 == 0 else nc.scalar
                q.dma_start(out_d[c][:, r0 * WP : (r0 + RT) * WP], dst)

            # -------- schedule ------------------------------------------
            # Main groups interleave with the SE chain from the very first
            # step (the SE chain alone stalls the PE ~4us on PSUM-drain
            # latencies): the six c=0 groups for tiles 0-5 run RAW (conv
            # drained to SBUF before any attention exists; a vector mulfix
            # applies the attention once sigmoided), everything else runs
            # fused. Each dense 18-MM main (~3.5us) separates every SE
            # producer from its consumer, hiding all drain/activation
            # latencies. SE deps: s(t) <- p(t) drains; q(t) <- relu of
            # s(t) AND s(t+1) (the dh=+1 row); r(t) <- q(t) drains.
            # mr = raw main (full tile), mh = raw main row-half (tiles 4/5
            # are halved so raw work is still available to fill the two
            # late-phase slots where no attention tile is sigmoided yet),
            # mf = fused main, x = mulfix. Every fused/x step sits AFTER
            # its r-step in program order (Tile deps follow program order).
            steps = [
                ("p", 0), ("p", 1), ("mr", 0, 0),
                ("s", 0), ("p", 2), ("mr", 1, 0),
                ("s", 1), ("mr", 2, 0), ("q", 0),
                ("p", 3), ("s", 2), ("mr", 3, 0),
                ("q", 1), ("r", 0), ("mf", 0, 1), ("x", 0),
                ("p", 4), ("s", 3), ("mh", 4, 0, 0),
                ("q", 2), ("r", 1), ("mf", 1, 1), ("x", 1),
                ("p", 5), ("s", 4), ("mh", 5, 0, 0),
                ("q", 3), ("r", 2), ("mf", 2, 1), ("x", 2),
                ("p", 6), ("s", 5), ("mh", 4, 0, 1),
                ("s", 6), ("q", 4), ("r", 3), ("mf", 3, 1), ("x", 3),
                ("mh", 5, 0, 1), ("q", 5),
                ("r", 4), ("mf", 4, 1), ("x", 4),
                ("q", 6), ("r", 5), ("mf", 5, 1), ("x", 5),
                ("r", 6), ("mf", 6, 0), ("mf", 6, 1),
            ]
            u1 = {}
            u2 = {}
            for step in steps:
                kind, t = step[0], step[1]
                if kind == "p":
                    u1[t] = conv1_pack(t)
                elif kind == "s":
                    conv1_sel(t, *u1[t])
                elif kind == "q":
                    u2[t] = conv2_pack(t)
                elif kind == "r":
                    conv2_sel(t, *u2[t])
                elif kind == "x":
                    mulfix(t, 0)
                elif kind == "mr":
                    main_raw(t, step[2])
                elif kind == "mh":
                    h = step[3]
                    main_raw(t, step[2], h * (RT // 2), (h + 1) * (RT // 2))
                else:
                    main_fused(t, step[2], split=(step == steps[-1]))

    nc.compile()
    return nc


_NC = None


def make_in_maps(x, weight, A_w, se_w1, se_w2):
    import ml_dtypes

    bf16 = ml_dtypes.bfloat16
    x = np.asarray(x, dtype=np.float32)
    # pre-padded x: [B, 128, ci-block, guard + 56*58 + guard] with zero pad
    # columns (c=0, c=57) and guards; ci-blocks side by side per partition
    xpad = np.zeros((B, 128, 2, HWP + 2), dtype=bf16)
    xv = xpad[:, :, :, 1 : 1 + HWP].reshape(B, 128, 2, H, WP)
    xv[:, :, :, :, 1 : W + 1] = (
        x.reshape(B, 2, 128, H, W).transpose(0, 2, 1, 3, 4).astype(bf16)
    )
    xpad = xpad.reshape(B, 128, 2 * (HWP + 2))

    # fold A_w into the conv weight on host (f32), then transpose+cast;
    # layout [co-block, 128ci, ci-block, 9, 128co] so each co-block half is
    # one contiguous DMA
    wm = np.asarray(weight, dtype=np.float32) * np.asarray(A_w, dtype=np.float32)
    wmT = wm.transpose(1, 2, 3, 0).reshape(2, 128, 9, 2, 128).astype(bf16)
    wmodT = np.ascontiguousarray(
        wmT.transpose(3, 1, 0, 2, 4).reshape(2, 128, 2 * 9 * 128)
    )

    # SE weights pre-packed: w1 kw groups at 32-col strides per kh slice
    # (block 0 cols [0:288), block 1 [288:576)), w2 separate on 16 parts
    w1T = np.asarray(se_w1, dtype=np.float32).transpose(1, 2, 3, 0)  # [ci,kh,kw,16]
    w1P = np.zeros((2, 128, 3, 3, 32), dtype=bf16)
    w1P[:, :, :, :, :CMID] = w1T.reshape(2, 128, 3, 3, CMID).astype(bf16)
    sew1P = np.ascontiguousarray(
        w1P.reshape(2, 128, 288).transpose(1, 0, 2).reshape(128, 2 * 288)
    )
    w2P = np.zeros((CMID, 3, 3, 32), dtype=bf16)
    w2P[:, :, :, :CMID] = (
        np.asarray(se_w2, dtype=np.float32)[0].astype(bf16)[:, :, :, None]
    )
    sew2P = np.ascontiguousarray(w2P.reshape(CMID, 288))

    in_maps = [
        {
            "xpad": np.ascontiguousarray(xpad[b]),
            "wmodT": wmodT,
            "sew1P": sew1P,
            "sew2P": sew2P,
        }
        for b in range(B)
    ]
    return in_maps


def kernel(x, weight, A_w, se_w1, se_w2):
    global _NC
    if _NC is None:
        _NC = build()
    in_maps = make_in_maps(x, weight, A_w, se_w1, se_w2)
    res = run_bass_kernel_spmd(_NC, in_maps, list(range(N_CORES)))
    out = np.stack([res.results[b]["outp"] for b in range(B)], axis=0)
    # strip pad columns: [B,2,128,56*58] -> [B,256,56,56]
    out = out.reshape(B, 2, 128, H, WP)[:, :, :, :, 1 : W + 1].reshape(B, C, H, W)
    return np.ascontiguousarray(out)

